# revision 24
# baseline (speedup 1.0000x reference)
"""DNC-LSTM encoder kernel for 8 TRN2 NeuronCores (self-contained).

Strategy (data-parallel over batch, 8 per core):
  Phase 1: on-device embedding gather (indirect DMA) + PE-transpose +
           big matmul X = emb @ W_x^T + b, staged to DRAM scratch in
           transposed (P-layout) blocks.
  Phase 2: 512-step recurrence. Gates computed as gates^T via
           stationary-weight matmuls (weights resident in SBUF, bf16),
           elementwise in P-layout, DNC memory bank per-example in
           B-layout ([8, 25, 6]) with broadcast-AP vector ops.
  Output h stored per 16 steps via PE transposes to [S, 8, 1024] slices.
"""

import numpy as np
import ml_dtypes

import concourse.bass as bass
import concourse.bacc as bacc
import concourse.mybir as mybir
import concourse.tile as tile
from concourse.bass_utils import run_bass_kernel_spmd
from concourse.masks import make_identity

# ---- problem constants (hardcoded per contest rules) ----
HID = 1024
EMB = 512
R = 4
NW = 25
WS = 6
IFACE = R * WS + R + 3 * WS + 1  # 47
VOCAB = 32000
SEQ = 512
BATCH = 64
NCORES = 8
BL = BATCH // NCORES  # 8 batch per core
NCH = HID // 128      # 8 hidden chunks
NMT = 1 + 4 * NCH     # 33 m-tiles: [iface, (i,f,o,g) x 8 chunks]
NKT_E = EMB // 128    # 4
NKT_H = HID // 128    # 8
NKT = NKT_E + NKT_H + 1  # 13 (last k-tile: r, 24 rows)
RW = R * WS           # 24
GROWS = NMT * 128     # 4224
KCOLS = NKT * 128     # 1664

F32 = mybir.dt.float32
BF16 = mybir.dt.float16  # compute dtype (fp16: 10-bit mantissa, 1 cyc/row)
I32 = mybir.dt.int32
AX = mybir.AxisListType
AF = mybir.ActivationFunctionType


def _bc(t, parts, dims, off=0):
    """Strided/broadcast AP from a tile (or AP) base; dims = [(step, count)]
    in elements. Partition step inherited from the base AP."""
    a = t[:] if not isinstance(t, bass.AP) else t
    return bass.AP(tensor=a.tensor, offset=a.offset + off,
                   ap=[[a.ap[0][0], parts]] + [list(d) for d in dims])


def _build(seq, sblk, tblk, dbg=False):
    nblk = seq // sblk
    SC = sblk * BL                 # X columns per s-block
    nsub = (sblk * BL) // 128      # gather chunks of 128 rows per block
    assert sblk * BL % 128 == 0 and seq % sblk == 0 and sblk % tblk == 0

    nc = bacc.Bacc("TRN2", target_bir_lowering=False, debug=False,
                   num_devices=NCORES)

    emb_t = nc.dram_tensor("emb", [VOCAB, EMB], F32, kind="ExternalInput")
    wt_in = nc.dram_tensor("wt", [128, NKT, NMT, 128], BF16, kind="ExternalInput")
    bv_in = nc.dram_tensor("bv", [128, NMT], F32, kind="ExternalInput")
    gidx = nc.dram_tensor("gidx", [nblk, 128, nsub], I32, kind="ExternalInput")
    out_t = nc.dram_tensor("out", [seq, BL, HID], F32, kind="ExternalOutput")
    dbg_t = None
    if dbg:
        dbg_t = nc.dram_tensor("dbg", [seq, BL, 384], F32, kind="ExternalOutput")
    xp = nc.dram_tensor("xp", [nblk, NMT, 128, SC], BF16, kind="Internal")

    with tile.TileContext(nc) as tc:
        with tc.tile_pool(name="const", bufs=1) as constp, \
             tc.tile_pool(name="w2", bufs=1) as w2p, \
             tc.tile_pool(name="state", bufs=1) as statep:

            ident = constp.tile([128, 128], F32)
            make_identity(nc, ident[:])
            eps12 = constp.tile([128, 1], F32)
            nc.vector.memset(eps12[:], 1e-12)

            MUL, ADD = mybir.AluOpType.mult, mybir.AluOpType.add

            # single ACT table (natural_log_exp): synthesize the rest
            def sigmoid_(out, in_):
                nc.scalar.activation(out, in_, AF.Exp, scale=-1.0)
                nc.vector.tensor_scalar_add(out, out, 1.0)
                nc.vector.reciprocal(out, out)

            def tanh_(out, in_):
                nc.scalar.activation(out, in_, AF.Exp, scale=2.0)
                nc.vector.tensor_scalar_add(out, out, 1.0)
                nc.vector.reciprocal(out, out)
                nc.vector.tensor_scalar(out, out, -2.0, 1.0, op0=MUL, op1=ADD)

            def beta_(out, in_):  # 1 + softplus(x)
                nc.scalar.activation(out, in_, AF.Exp)
                nc.vector.tensor_scalar_add(out, out, 1.0)
                nc.scalar.activation(out, out, AF.Ln)
                nc.vector.tensor_scalar_add(out, out, 1.0)

            def rsqrt_(out, in_):  # 1/sqrt(x + 1e-12)
                nc.scalar.activation(out, in_, AF.Ln, bias=eps12[0:BL, :])
                nc.scalar.activation(out, out, AF.Exp, scale=-0.5)
            b_sb = constp.tile([128, NMT], F32)
            nc.sync.dma_start(out=b_sb[:], in_=bv_in[:])

            # persistent phase-2 weights: k-tiles 4..12 (h and r)
            wt2 = w2p.tile([128, NKT_H + 1, NMT, 128], BF16)
            nc.sync.dma_start(out=wt2[:], in_=wt_in[:, NKT_E:NKT, :, :])

            c_t = statep.tile([128, NCH, BL], F32)
            nc.vector.memset(c_t[:], 0.0)
            M_t = statep.tile([BL, NW, WS], F32)
            nc.vector.memset(M_t[:], 0.0)
            hT0 = statep.tile([128, NCH, BL], BF16)
            nc.vector.memset(hT0[:], 0.0)
            rT0 = statep.tile([RW, BL], BF16)
            nc.vector.memset(rT0[:], 0.0)

            # ---------------- PHASE 1: X projection ----------------
            with tc.tile_pool(name="wx", bufs=1) as wxp, \
                 tc.tile_pool(name="gth", bufs=3) as gthp, \
                 tc.tile_pool(name="embT", bufs=2) as embTp, \
                 tc.tile_pool(name="idx", bufs=2) as idxp, \
                 tc.tile_pool(name="xo", bufs=3) as xop, \
                 tc.tile_pool(name="ps1", bufs=2, space="PSUM") as ps1p, \
                 tc.tile_pool(name="pst", bufs=2, space="PSUM") as pstp:

                wx = wxp.tile([128, NKT_E, NMT, 128], BF16)
                nc.sync.dma_start(out=wx[:], in_=wt_in[:, 0:NKT_E, :, :])

                for blk in range(nblk):
                    idx_t = idxp.tile([128, nsub], I32)
                    nc.sync.dma_start(out=idx_t[:], in_=gidx[blk])
                    embT = embTp.tile([128, NKT_E, SC], BF16)
                    for sub in range(nsub):
                        g = gthp.tile([128, EMB], F32)
                        nc.gpsimd.indirect_dma_start(
                            out=g[:], out_offset=None, in_=emb_t[:],
                            in_offset=bass.IndirectOffsetOnAxis(
                                ap=idx_t[:, sub:sub + 1], axis=0))
                        for k in range(NKT_E):
                            tp = pstp.tile([128, 128], F32, tag="tp1")
                            nc.tensor.transpose(tp[:], g[:, 128 * k:128 * (k + 1)],
                                                ident[:])
                            nc.vector.tensor_copy(
                                embT[:, k, 128 * sub:128 * (sub + 1)], tp[:])
                    for m in range(NMT):
                        ps = ps1p.tile([128, SC], F32, tag="ps1")
                        for k in range(NKT_E):
                            nc.tensor.matmul(ps[:], wx[:, k, m, :], embT[:, k, :],
                                             start=(k == 0), stop=(k == NKT_E - 1))
                        xo = xop.tile([128, SC], BF16)
                        nc.vector.tensor_scalar_add(xo[:], ps[:], b_sb[:, m:m + 1])
                        nc.sync.dma_start(out=xp[blk, m], in_=xo[:])

            tc.strict_bb_all_engine_barrier()

            # ---------------- PHASE 2: recurrence ----------------
            with tc.tile_pool(name="xb", bufs=2) as xbp, \
                 tc.tile_pool(name="hT", bufs=2) as hTp, \
                 tc.tile_pool(name="rT", bufs=2) as rTp, \
                 tc.tile_pool(name="stg", bufs=2) as stgp, \
                 tc.tile_pool(name="ew", bufs=2) as ewp, \
                 tc.tile_pool(name="dnc", bufs=2) as dncp, \
                 tc.tile_pool(name="ob", bufs=2) as obp, \
                 tc.tile_pool(name="pIF", bufs=1, space="PSUM") as pIFp, \
                 tc.tile_pool(name="rc", bufs=1, space="PSUM") as rconp, \
                 tc.tile_pool(name="pA", bufs=2, space="PSUM") as pAp, \
                 tc.tile_pool(name="pB", bufs=2, space="PSUM") as pBp, \
                 tc.tile_pool(name="ptr", bufs=2, space="PSUM") as ptrp:

                def load_xblk(blk):
                    xb = xbp.tile([128, NMT, SC], BF16, tag="xb")
                    src = bass.AP(tensor=xp[:].tensor,
                                  offset=blk * NMT * 128 * SC,
                                  ap=[[SC, 128], [128 * SC, NMT], [1, SC]])
                    nc.sync.dma_start(out=xb[:], in_=src)
                    return xb

                xb_cur = load_xblk(0)
                xb_next = None
                hT_prev, rT_prev = hT0, rT0
                stage = None

                for t in range(seq):
                    blk, soff = t // sblk, t % sblk
                    toff = t % tblk
                    if toff == 0:
                        stage = stgp.tile([128, NCH, tblk, BL], F32, tag="stg")
                    if soff == sblk // 2 and blk + 1 < nblk:
                        xb_next = load_xblk(blk + 1)
                    xcol = soff * BL

                    # --- gate m-tiles, two halves of 4 chunks each ---
                    hT_new = hTp.tile([128, NCH, BL], BF16, tag="hT")
                    rcon = rconp.tile([128, 32, BL], F32, tag="rcon")
                    for half in range(2):
                        pp = (pAp if half == 0 else pBp)
                        ps = pp.tile([128, 16, BL], F32, tag=f"pg{half}")
                        for j4 in range(4):
                            for gi in range(4):
                                m = 1 + (half * 4 + j4) * 4 + gi
                                for k in range(NKT_H):
                                    nc.tensor.matmul(
                                        ps[:, j4 * 4 + gi, :],
                                        wt2[:, k, m, :], hT_prev[:, k, :],
                                        start=(k == 0), stop=(k == NKT_H - 1))
                        # r contribution in its own psum tile (closed 1-MM
                        # groups), available well after DNC of t-1 finishes
                        for j4 in range(4):
                            for gi in range(4):
                                m = 1 + (half * 4 + j4) * 4 + gi
                                nc.tensor.matmul(
                                    rcon[:, half * 16 + j4 * 4 + gi, :],
                                    wt2[0:RW, NKT_H, m, :], rT_prev[:],
                                    start=True, stop=True)
                        mlo = 1 + 16 * half
                        rsb = ewp.tile([128, 16, BL], F32, tag=f"rsb{half}")
                        nc.vector.scalar_tensor_tensor(
                            rsb[:], rcon[:, half * 16:(half + 1) * 16, :], 1.0,
                            xb_cur[:, mlo:mlo + 16, xcol:xcol + BL],
                            op0=MUL, op1=ADD)
                        nc.vector.tensor_add(ps[:], ps[:], rsb[:])
                        acts = ewp.tile([128, 4, 4, BL], F32, tag=f"acts{half}")
                        psb = ps[:].rearrange("p (j g) b -> p j g b", j=4, g=4)
                        sigmoid_(acts[:, :, 0:3, :], psb[:, :, 0:3, :])
                        tanh_(acts[:, :, 3:4, :], psb[:, :, 3:4, :])
                        cs = c_t[:, half * 4:(half + 1) * 4, :]
                        tmp = dncp.tile([128, 4, BL], F32, tag=f"ig{half}")
                        nc.vector.tensor_mul(tmp[:], acts[:, :, 0, :],
                                             acts[:, :, 3, :])
                        nc.vector.tensor_mul(cs, acts[:, :, 1, :], cs)
                        nc.vector.tensor_add(cs, cs, tmp[:])
                        tch = dncp.tile([128, 4, BL], F32, tag=f"tc{half}")
                        tanh_(tch[:], cs)
                        hslice = stage[:, half * 4:(half + 1) * 4, toff, :]
                        nc.vector.tensor_mul(hslice, acts[:, :, 2, :], tch[:])
                        nc.vector.tensor_copy(hT_new[:, half * 4:(half + 1) * 4, :],
                                              hslice)

                    # --- iface m-tile (m=0) ---
                    psIF = pIFp.tile([128, BL], F32, tag="pIF")
                    for k in range(NKT_H):
                        nc.tensor.matmul(psIF[:], wt2[:, k, 0, :], hT_new[:, k, :],
                                         start=(k == 0), stop=(k == NKT_H - 1))
                    nc.vector.tensor_add(psIF[:], psIF[:],
                                         xb_cur[:, 0, xcol:xcol + BL])

                    # --- DNC chain (consumes iface, produces rT_new) ---
                    ifc_sb = dncp.tile([128, BL], F32, tag="ifcsb")
                    nc.vector.tensor_copy(ifc_sb[:], psIF[:])
                    ifcT_ps = ptrp.tile([BL, 128], F32, tag="tp")
                    nc.tensor.transpose(ifcT_ps[:], ifc_sb[:], ident[:])
                    ifc = dncp.tile([BL, 128], F32, tag="ifc")
                    nc.scalar.copy(ifc[:], ifcT_ps[:])
                    # iface cols: rk 0:24, rb 24:28, wk 28:34, wb 34:35,
                    #             er 35:41, wv 41:47
                    rbeta = dncp.tile([BL, R], F32, tag="rbeta")
                    beta_(rbeta[:], ifc[:, 24:28])
                    wbeta = dncp.tile([BL, 1], F32, tag="wbeta")
                    beta_(wbeta[:], ifc[:, 34:35])
                    erase = dncp.tile([BL, WS], F32, tag="erase")
                    sigmoid_(erase[:], ifc[:, 35:41])

                    # pre-write M norms
                    msq = dncp.tile([BL, NW, WS], F32, tag="msq")
                    nc.vector.tensor_mul(msq[:], M_t[:], M_t[:])
                    mnr = dncp.tile([BL, NW], F32, tag="mnr")
                    nc.vector.reduce_sum(mnr[:], msq[:], axis=AX.X)
                    rnM = dncp.tile([BL, NW], F32, tag="rnM")
                    rsqrt_(rnM[:], mnr[:])
                    # wk norm
                    wks = dncp.tile([BL, WS], F32, tag="wks")
                    nc.vector.tensor_mul(wks[:], ifc[:, 28:34], ifc[:, 28:34])
                    wkn = dncp.tile([BL, 1], F32, tag="wkn")
                    nc.vector.reduce_sum(wkn[:], wks[:], axis=AX.X)
                    rnK = dncp.tile([BL, 1], F32, tag="rnK")
                    rsqrt_(rnK[:], wkn[:])
                    # write similarity + softmax (no max-sub; |z| bounded)
                    dp = dncp.tile([BL, NW, WS], F32, tag="dp")
                    nc.vector.tensor_mul(dp[:], M_t[:],
                                         _bc(ifc, BL, [(0, NW), (1, WS)], off=28))
                    dot = dncp.tile([BL, NW], F32, tag="dot")
                    nc.vector.reduce_sum(dot[:], dp[:], axis=AX.X)
                    nc.vector.tensor_mul(dot[:], dot[:], rnM[:])
                    nc.vector.tensor_scalar_mul(dot[:], dot[:], rnK[:])
                    ex = dncp.tile([BL, NW], F32, tag="ex")
                    exs = dncp.tile([BL, 1], F32, tag="exs")
                    nc.scalar.activation(ex[:], dot[:], AF.Exp, scale=wbeta[:],
                                         accum_out=exs[:])
                    rZ = dncp.tile([BL, 1], F32, tag="rZ")
                    nc.vector.reciprocal(rZ[:], exs[:])
                    ww = dncp.tile([BL, NW], F32, tag="ww")
                    nc.vector.tensor_scalar_mul(ww[:], ex[:], rZ[:])
                    # memory write: M = M*(1 - ww x er) + ww x wv
                    ewt = ewp.tile([BL, NW, WS], F32, tag="ewt")
                    nc.vector.tensor_mul(ewt[:], _bc(ww, BL, [(1, NW), (0, WS)]),
                                         _bc(erase, BL, [(0, NW), (1, WS)]))
                    nc.vector.tensor_scalar(ewt[:], ewt[:], -1.0, 1.0,
                                            op0=mybir.AluOpType.mult,
                                            op1=mybir.AluOpType.add)
                    nc.vector.tensor_mul(M_t[:], M_t[:], ewt[:])
                    wwv = ewp.tile([BL, NW, WS], F32, tag="wwv")
                    nc.vector.tensor_mul(wwv[:], _bc(ww, BL, [(1, NW), (0, WS)]),
                                         _bc(ifc, BL, [(0, NW), (1, WS)], off=41))
                    nc.vector.tensor_add(M_t[:], M_t[:], wwv[:])
                    # post-write norms
                    nc.vector.tensor_mul(msq[:], M_t[:], M_t[:])
                    mnr2 = dncp.tile([BL, NW], F32, tag="mnr2")
                    nc.vector.reduce_sum(mnr2[:], msq[:], axis=AX.X)
                    rnM2 = dncp.tile([BL, NW], F32, tag="rnM2")
                    rsqrt_(rnM2[:], mnr2[:])
                    # read keys norms
                    rks = dncp.tile([BL, R, WS], F32, tag="rks")
                    nc.vector.tensor_mul(rks[:], ifc[:, 0:24].rearrange("p (r w) -> p r w", r=R),
                                         ifc[:, 0:24].rearrange("p (r w) -> p r w", r=R))
                    rkn = dncp.tile([BL, R], F32, tag="rkn")
                    nc.vector.reduce_sum(rkn[:], rks[:], axis=AX.X)
                    rnR = dncp.tile([BL, R], F32, tag="rnR")
                    rsqrt_(rnR[:], rkn[:])
                    nc.vector.tensor_mul(rnR[:], rnR[:], rbeta[:])  # fold beta
                    # read scores [b, r, n]
                    pr2 = ewp.tile([BL, R, NW, WS], F32, tag="pr2")
                    nc.vector.tensor_mul(
                        pr2[:],
                        _bc(ifc, BL, [(WS, R), (0, NW), (1, WS)], off=0),
                        _bc(M_t, BL, [(0, R), (WS, NW), (1, WS)]))
                    sc = dncp.tile([BL, R, NW], F32, tag="sc")
                    nc.vector.reduce_sum(sc[:], pr2[:], axis=AX.X)
                    nc.vector.tensor_mul(sc[:], sc[:],
                                         _bc(rnM2, BL, [(0, R), (1, NW)]))
                    nc.vector.tensor_mul(sc[:], sc[:],
                                         _bc(rnR, BL, [(1, R), (0, NW)]))
                    e2 = dncp.tile([BL, R, NW], F32, tag="e2")
                    nc.scalar.activation(e2[:], sc[:], AF.Exp)
                    s2 = dncp.tile([BL, R], F32, tag="s2")
                    nc.vector.reduce_sum(s2[:], e2[:], axis=AX.X)
                    rS = dncp.tile([BL, R], F32, tag="rS")
                    nc.vector.reciprocal(rS[:], s2[:])
                    nc.vector.tensor_mul(e2[:], e2[:],
                                         _bc(rS, BL, [(1, R), (0, NW)]))
                    # r_vec [b, r, w] = sum_n w_r * M
                    pr3 = ewp.tile([BL, R, WS, NW], F32, tag="pr3")
                    nc.vector.tensor_mul(
                        pr3[:],
                        _bc(e2, BL, [(NW, R), (0, WS), (1, NW)]),
                        _bc(M_t, BL, [(0, R), (1, WS), (WS, NW)]))
                    rvec = dncp.tile([BL, RW], F32, tag="rvec")
                    nc.vector.reduce_sum(rvec[:], pr3[:], axis=AX.X)
                    rT_ps = ptrp.tile([RW, BL], F32, tag="tp")
                    nc.tensor.transpose(rT_ps[:], rvec[:], ident[:BL, :BL])
                    rT_new = rTp.tile([RW, BL], BF16, tag="rT")
                    nc.vector.tensor_copy(rT_new[:], rT_ps[:])
                    if dbg:
                        db = dncp.tile([BL, 384], F32, tag="db")
                        nc.vector.memset(db[:], 0.0)
                        nc.vector.tensor_copy(db[:, 328:336], ifc_sb[0:BL, :])
                        nc.vector.tensor_copy(db[:, 336:344], psIF[0:BL, :])
                        nc.vector.tensor_copy(db[:, 0:128], ifc[:])
                        nc.vector.tensor_copy(db[:, 128:153], ww[:])
                        nc.vector.tensor_copy(db[:, 153:177], rvec[:])
                        nc.vector.tensor_copy(db[:, 177:327],
                                              M_t[:].rearrange("p n w -> p (n w)"))
                        nc.sync.dma_start(out=dbg_t[t], in_=db[:])

                    hT_prev, rT_prev = hT_new, rT_new

                    # --- output staging flush every tblk steps ---
                    if toff == tblk - 1:
                        t0 = t - tblk + 1
                        for j in range(NCH):
                            otp = ptrp.tile([128, 128], F32, tag="tp")
                            nc.tensor.transpose(
                                otp[0:tblk * BL, :],
                                _bc(stage, 128, [(1, tblk * BL)],
                                    off=j * tblk * BL), ident[:])
                            ob = obp.tile([tblk * BL, 128], F32, tag="ob")
                            nc.scalar.copy(ob[:], otp[0:tblk * BL, :])
                            nc.sync.dma_start(
                                out=out_t[t0:t0 + tblk, :, 128 * j:128 * (j + 1)],
                                in_=ob[:])
                    if soff == sblk - 1 and blk + 1 < nblk:
                        xb_cur = xb_next

    nc.compile()
    return nc


_CACHE = {}


def _get_nc(seq, sblk, tblk):
    key = (seq, sblk, tblk)
    if key not in _CACHE:
        _CACHE[key] = _build(seq, sblk, tblk)
    return _CACHE[key]


def _prep_weights(W_ih, W_hh, b, W_if, b_if):
    W_ih = np.asarray(W_ih, np.float32)
    W_hh = np.asarray(W_hh, np.float32)
    b = np.asarray(b, np.float32)
    W_if = np.asarray(W_if, np.float32)
    b_if = np.asarray(b_if, np.float32)
    bigW = np.zeros((GROWS, KCOLS), np.float32)
    bigb = np.zeros((GROWS,), np.float32)
    # m=0: iface rows (h part only)
    bigW[0:IFACE, 512:1536] = W_if
    bigb[0:IFACE] = b_if
    # gate tiles: order (i, f, o, g) per hidden chunk
    gate_off = {0: 0, 1: HID, 2: 3 * HID, 3: 2 * HID}  # i, f, o, g
    for j in range(NCH):
        for gi in range(4):
            m = 1 + 4 * j + gi
            rows = slice(gate_off[gi] + 128 * j, gate_off[gi] + 128 * j + 128)
            bigW[128 * m:128 * m + 128, 0:512] = W_ih[rows, 0:512]
            bigW[128 * m:128 * m + 128, 512:1536] = W_hh[rows, :]
            bigW[128 * m:128 * m + 128, 1536:1536 + RW] = W_ih[rows, 512:512 + RW]
            bigb[128 * m:128 * m + 128] = b[rows]
    wt_host = np.ascontiguousarray(
        bigW.reshape(NMT, 128, NKT, 128).transpose(3, 2, 0, 1)
    ).astype(np.float16)
    bv_host = np.ascontiguousarray(bigb.reshape(NMT, 128).T)
    return wt_host, bv_host


def kernel(src, enc_pad_ix, emb_table, W_ih, W_hh, b, W_if, b_if,
           _seq=None, _sblk=64, _tblk=16, _trace=False):
    src = np.asarray(src)
    seq = src.shape[0] if _seq is None else _seq
    emb_np = np.asarray(emb_table, np.float32)
    wt_host, bv_host = _prep_weights(W_ih, W_hh, b, W_if, b_if)

    nblk = seq // _sblk
    nsub = (_sblk * BL) // 128
    in_maps = []
    for c in range(NCORES):
        toks = np.ascontiguousarray(
            src[:seq, 8 * c:8 * c + 8].astype(np.int32)
            .reshape(nblk, _sblk * BL)
            .reshape(nblk, nsub, 128)
            .transpose(0, 2, 1))
        in_maps.append({"emb": emb_np, "wt": wt_host, "bv": bv_host,
                        "gidx": np.ascontiguousarray(toks)})

    nc = _get_nc(seq, _sblk, _tblk)
    import time as _time
    _t0 = _time.time()
    try:
        res = run_bass_kernel_spmd(nc, in_maps, core_ids=list(range(NCORES)),
                                   trace=_trace)
    except ModuleNotFoundError:
        res = run_bass_kernel_spmd(nc, in_maps, core_ids=list(range(NCORES)),
                                   trace=False)
    kernel._last_run_wall_s = _time.time() - _t0

    outputs = np.empty((seq, BATCH, HID), np.float32)
    for c in range(NCORES):
        outputs[:, 8 * c:8 * c + 8, :] = res.results[c]["out"]
    lengths = np.minimum((src[:seq] != int(enc_pad_ix)).sum(axis=0), seq - 1)
    last_state = outputs[lengths, np.arange(BATCH)]
    kernel._last_exec_time_ns = res.exec_time_ns
    return outputs, last_state


# revision 26
# speedup vs baseline: 2.6281x; 2.6281x over previous
"""DNC-LSTM encoder kernel for 8 TRN2 NeuronCores (self-contained).

Strategy (data-parallel over batch, 8 per core):
  Phase 1: on-device embedding gather (indirect DMA) + PE-transpose +
           big matmul X = emb @ W_x^T + b, staged to DRAM scratch in
           transposed (P-layout) blocks.
  Phase 2: 512-step recurrence. Gates computed as gates^T via
           stationary-weight matmuls (weights resident in SBUF, bf16),
           elementwise in P-layout, DNC memory bank per-example in
           B-layout ([8, 25, 6]) with broadcast-AP vector ops.
  Output h stored per 16 steps via PE transposes to [S, 8, 1024] slices.
"""

import hashlib
import os
import shutil

import numpy as np

import concourse.bass as bass
import concourse.bacc as bacc
import concourse.mybir as mybir
import concourse.tile as tile
from concourse import bass2jax as _b2j
from concourse import bass_utils as _bu
from concourse.bass_utils import run_bass_kernel_spmd
from concourse.masks import make_identity


def _install_neff_cache():
    # content-addressed NEFF cache: identical BIR -> skip walrus (~30s)
    orig = _bu.compile_bir_kernel
    if getattr(orig, "_neff_cached", False):
        return
    cdir = "/var/tmp/bass_neff_cache"

    def cached(bir_json, tmpdir, neff_name="file.neff"):
        try:
            os.makedirs(cdir, exist_ok=True)
            h = hashlib.sha256(bir_json).hexdigest()
            cpath = os.path.join(cdir, h + ".neff")
            if os.path.exists(cpath):
                out = os.path.join(tmpdir, neff_name)
                shutil.copyfile(cpath, out)
                return out
        except OSError:
            return orig(bir_json, tmpdir, neff_name=neff_name)
        res = orig(bir_json, tmpdir, neff_name=neff_name)
        try:
            shutil.copyfile(res, cpath + ".tmp")
            os.replace(cpath + ".tmp", cpath)
        except OSError:
            pass
        return res

    cached._neff_cached = True
    _bu.compile_bir_kernel = cached
    if getattr(_b2j, "compile_bir_kernel", None) is orig:
        _b2j.compile_bir_kernel = cached


_install_neff_cache()

# ---- problem constants (hardcoded per contest rules) ----
HID = 1024
EMB = 512
R = 4
NW = 25
WS = 6
IFACE = R * WS + R + 3 * WS + 1  # 47
VOCAB = 32000
SEQ = 512
BATCH = 64
NCORES = 8
BL = BATCH // NCORES  # 8 batch per core
NCH = HID // 128      # 8 hidden chunks
NMT = 1 + 4 * NCH     # 33 m-tiles: [iface, (i,f,o,g) x 8 chunks]
NKT_E = EMB // 128    # 4
NKT_H = HID // 128    # 8
NKT = NKT_E + NKT_H + 1  # 13 (last k-tile: r, 24 rows)
RW = R * WS           # 24
GROWS = NMT * 128     # 4224
KCOLS = NKT * 128     # 1664

F32 = mybir.dt.float32
BF16 = mybir.dt.float16  # compute dtype (fp16: 10-bit mantissa, 1 cyc/row)
I32 = mybir.dt.int32
AX = mybir.AxisListType
AF = mybir.ActivationFunctionType


def _bc(t, parts, dims, off=0):
    """Strided/broadcast AP from a tile (or AP) base; dims = [(step, count)]
    in elements. Partition step inherited from the base AP."""
    a = t[:] if not isinstance(t, bass.AP) else t
    return bass.AP(tensor=a.tensor, offset=a.offset + off,
                   ap=[[a.ap[0][0], parts]] + [list(d) for d in dims])


def _build(seq, sblk, tblk, dbg=False):
    nblk = seq // sblk
    SC = sblk * BL                 # X columns per s-block
    nsub = (sblk * BL) // 128      # gather chunks of 128 rows per block
    assert sblk * BL % 128 == 0 and seq % sblk == 0 and sblk % tblk == 0

    nc = bacc.Bacc("TRN2", target_bir_lowering=False, debug=False,
                   num_devices=NCORES)

    emb_t = nc.dram_tensor("emb", [VOCAB, EMB], BF16, kind="ExternalInput")
    wt_in = nc.dram_tensor("wt", [128, NKT, NMT, 128], BF16, kind="ExternalInput")
    bv_in = nc.dram_tensor("bv", [128, NMT], F32, kind="ExternalInput")
    gidx = nc.dram_tensor("gidx", [nblk, 128, nsub], I32, kind="ExternalInput")
    out_t = nc.dram_tensor("out", [seq, BL, HID], F32, kind="ExternalOutput")
    dbg_t = None
    if dbg:
        dbg_t = nc.dram_tensor("dbg", [seq, BL, 384], F32, kind="ExternalOutput")
    xp = nc.dram_tensor("xp", [nblk, NMT, 128, SC], BF16, kind="Internal")

    with tile.TileContext(nc) as tc:
        with tc.tile_pool(name="const", bufs=1) as constp, \
             tc.tile_pool(name="w2", bufs=1) as w2p, \
             tc.tile_pool(name="state", bufs=1) as statep:

            ident = constp.tile([128, 128], F32)
            make_identity(nc, ident[:])
            identh = constp.tile([128, 128], BF16)
            nc.vector.tensor_copy(identh[:], ident[:])
            eps12 = constp.tile([128, 1], F32)
            nc.vector.memset(eps12[:], 1e-12)

            MUL, ADD = mybir.AluOpType.mult, mybir.AluOpType.add

            # single ACT table (natural_log_exp): synthesize the rest
            def sigmoid_(out, in_):
                nc.scalar.activation(out, in_, AF.Exp, scale=-1.0)
                nc.vector.tensor_scalar_add(out, out, 1.0)
                nc.vector.reciprocal(out, out)

            def tanh_(out, in_):
                nc.scalar.activation(out, in_, AF.Exp, scale=2.0)
                nc.vector.tensor_scalar_add(out, out, 1.0)
                nc.vector.reciprocal(out, out)
                nc.vector.tensor_scalar(out, out, -2.0, 1.0, op0=MUL, op1=ADD)

            def beta_(out, in_):  # 1 + softplus(x)
                nc.scalar.activation(out, in_, AF.Exp)
                nc.vector.tensor_scalar_add(out, out, 1.0)
                nc.scalar.activation(out, out, AF.Ln)
                nc.vector.tensor_scalar_add(out, out, 1.0)

            def rsqrt_(out, in_):  # 1/sqrt(x + 1e-12)
                nc.scalar.activation(out, in_, AF.Ln, bias=eps12[0:BL, :])
                nc.scalar.activation(out, out, AF.Exp, scale=-0.5)
            b_sb = constp.tile([128, NMT], F32)
            nc.sync.dma_start(out=b_sb[:], in_=bv_in[:])

            # persistent phase-2 weights: k-tiles 4..12 (h and r)
            wt2 = w2p.tile([128, NKT_H + 1, NMT, 128], BF16)
            nc.sync.dma_start(out=wt2[:], in_=wt_in[:, NKT_E:NKT, :, :])

            c_t = statep.tile([128, NCH, BL], F32)
            nc.vector.memset(c_t[:], 0.0)
            M_t = statep.tile([BL, NW, WS], F32)
            nc.vector.memset(M_t[:], 0.0)
            hT0 = statep.tile([128, NCH, BL], BF16)
            nc.vector.memset(hT0[:], 0.0)
            rT0 = statep.tile([RW, BL], BF16)
            nc.vector.memset(rT0[:], 0.0)

            # ---------------- PHASE 1: X projection ----------------
            with tc.tile_pool(name="wx", bufs=1) as wxp, \
                 tc.tile_pool(name="gth", bufs=3) as gthp, \
                 tc.tile_pool(name="embT", bufs=2) as embTp, \
                 tc.tile_pool(name="idx", bufs=2) as idxp, \
                 tc.tile_pool(name="xo", bufs=3) as xop, \
                 tc.tile_pool(name="ps1", bufs=2, space="PSUM") as ps1p, \
                 tc.tile_pool(name="pst", bufs=2, space="PSUM") as pstp:

                wx = wxp.tile([128, NKT_E, NMT, 128], BF16)
                nc.sync.dma_start(out=wx[:], in_=wt_in[:, 0:NKT_E, :, :])

                for blk in range(nblk):
                    idx_t = idxp.tile([128, nsub], I32)
                    nc.sync.dma_start(out=idx_t[:], in_=gidx[blk])
                    embT = embTp.tile([128, NKT_E, SC], BF16)
                    for sub in range(nsub):
                        g = gthp.tile([128, EMB], BF16)
                        nc.gpsimd.indirect_dma_start(
                            out=g[:], out_offset=None, in_=emb_t[:],
                            in_offset=bass.IndirectOffsetOnAxis(
                                ap=idx_t[:, sub:sub + 1], axis=0))
                        for k in range(NKT_E):
                            tp = pstp.tile([128, 128], BF16, tag="tp1")
                            nc.tensor.transpose(tp[:], g[:, 128 * k:128 * (k + 1)],
                                                identh[:])
                            nc.vector.tensor_copy(
                                embT[:, k, 128 * sub:128 * (sub + 1)], tp[:])
                    for m in range(NMT):
                        ps = ps1p.tile([128, SC], F32, tag="ps1")
                        for k in range(NKT_E):
                            nc.tensor.matmul(ps[:], wx[:, k, m, :], embT[:, k, :],
                                             start=(k == 0), stop=(k == NKT_E - 1))
                        xo = xop.tile([128, SC], BF16)
                        nc.vector.tensor_scalar_add(xo[:], ps[:], b_sb[:, m:m + 1])
                        nc.sync.dma_start(out=xp[blk, m], in_=xo[:])

            tc.strict_bb_all_engine_barrier()

            # ---------------- PHASE 2: recurrence ----------------
            with tc.tile_pool(name="xb", bufs=2) as xbp, \
                 tc.tile_pool(name="hT", bufs=2) as hTp, \
                 tc.tile_pool(name="rT", bufs=2) as rTp, \
                 tc.tile_pool(name="stg", bufs=2) as stgp, \
                 tc.tile_pool(name="ew", bufs=2) as ewp, \
                 tc.tile_pool(name="dnc", bufs=2) as dncp, \
                 tc.tile_pool(name="ob", bufs=2) as obp, \
                 tc.tile_pool(name="pIF", bufs=1, space="PSUM") as pIFp, \
                 tc.tile_pool(name="rc", bufs=1, space="PSUM") as rconp, \
                 tc.tile_pool(name="pA", bufs=2, space="PSUM") as pAp, \
                 tc.tile_pool(name="pB", bufs=2, space="PSUM") as pBp, \
                 tc.tile_pool(name="ptr", bufs=2, space="PSUM") as ptrp:

                def load_xblk(blk):
                    xb = xbp.tile([128, NMT, SC], BF16, tag="xb")
                    src = bass.AP(tensor=xp[:].tensor,
                                  offset=blk * NMT * 128 * SC,
                                  ap=[[SC, 128], [128 * SC, NMT], [1, SC]])
                    nc.sync.dma_start(out=xb[:], in_=src)
                    return xb

                xb_cur = load_xblk(0)
                xb_next = None
                hT_prev, rT_prev = hT0, rT0
                stage = None

                for t in range(seq):
                    blk, soff = t // sblk, t % sblk
                    toff = t % tblk
                    if toff == 0:
                        stage = stgp.tile([128, NCH, tblk, BL], F32, tag="stg")
                    if soff == sblk // 2 and blk + 1 < nblk:
                        xb_next = load_xblk(blk + 1)
                    xcol = soff * BL

                    # --- gate m-tiles, two halves of 4 chunks each ---
                    hT_new = hTp.tile([128, NCH, BL], BF16, tag="hT")
                    rcon = rconp.tile([128, 32, BL], F32, tag="rcon")
                    for half in range(2):
                        pp = (pAp if half == 0 else pBp)
                        ps = pp.tile([128, 16, BL], F32, tag=f"pg{half}")
                        for j4 in range(4):
                            for gi in range(4):
                                m = 1 + (half * 4 + j4) * 4 + gi
                                for k in range(NKT_H):
                                    nc.tensor.matmul(
                                        ps[:, j4 * 4 + gi, :],
                                        wt2[:, k, m, :], hT_prev[:, k, :],
                                        start=(k == 0), stop=(k == NKT_H - 1))
                        # r contribution in its own psum tile (closed 1-MM
                        # groups), available well after DNC of t-1 finishes
                        for j4 in range(4):
                            for gi in range(4):
                                m = 1 + (half * 4 + j4) * 4 + gi
                                nc.tensor.matmul(
                                    rcon[:, half * 16 + j4 * 4 + gi, :],
                                    wt2[0:RW, NKT_H, m, :], rT_prev[:],
                                    start=True, stop=True)
                        mlo = 1 + 16 * half
                        rsb = ewp.tile([128, 16, BL], F32, tag=f"rsb{half}")
                        nc.vector.scalar_tensor_tensor(
                            rsb[:], rcon[:, half * 16:(half + 1) * 16, :], 1.0,
                            xb_cur[:, mlo:mlo + 16, xcol:xcol + BL],
                            op0=MUL, op1=ADD)
                        nc.vector.tensor_add(ps[:], ps[:], rsb[:])
                        acts = ewp.tile([128, 4, 4, BL], F32, tag=f"acts{half}")
                        psb = ps[:].rearrange("p (j g) b -> p j g b", j=4, g=4)
                        sigmoid_(acts[:, :, 0:3, :], psb[:, :, 0:3, :])
                        tanh_(acts[:, :, 3:4, :], psb[:, :, 3:4, :])
                        cs = c_t[:, half * 4:(half + 1) * 4, :]
                        tmp = dncp.tile([128, 4, BL], F32, tag=f"ig{half}")
                        nc.vector.tensor_mul(tmp[:], acts[:, :, 0, :],
                                             acts[:, :, 3, :])
                        nc.vector.tensor_mul(cs, acts[:, :, 1, :], cs)
                        nc.vector.tensor_add(cs, cs, tmp[:])
                        tch = dncp.tile([128, 4, BL], F32, tag=f"tc{half}")
                        tanh_(tch[:], cs)
                        hslice = stage[:, half * 4:(half + 1) * 4, toff, :]
                        nc.vector.tensor_mul(hslice, acts[:, :, 2, :], tch[:])
                        nc.vector.tensor_copy(hT_new[:, half * 4:(half + 1) * 4, :],
                                              hslice)

                    # --- iface m-tile (m=0) ---
                    psIF = pIFp.tile([128, BL], F32, tag="pIF")
                    for k in range(NKT_H):
                        nc.tensor.matmul(psIF[:], wt2[:, k, 0, :], hT_new[:, k, :],
                                         start=(k == 0), stop=(k == NKT_H - 1))
                    nc.vector.tensor_add(psIF[:], psIF[:],
                                         xb_cur[:, 0, xcol:xcol + BL])

                    # --- DNC chain (consumes iface, produces rT_new) ---
                    ifc_sb = dncp.tile([128, BL], F32, tag="ifcsb")
                    nc.vector.tensor_copy(ifc_sb[:], psIF[:])
                    ifcT_ps = ptrp.tile([BL, 128], F32, tag="tp")
                    nc.tensor.transpose(ifcT_ps[:], ifc_sb[:], ident[:])
                    ifc = dncp.tile([BL, 128], F32, tag="ifc")
                    nc.scalar.copy(ifc[:], ifcT_ps[:])
                    # iface cols: rk 0:24, rb 24:28, wk 28:34, wb 34:35,
                    #             er 35:41, wv 41:47
                    rbeta = dncp.tile([BL, R], F32, tag="rbeta")
                    beta_(rbeta[:], ifc[:, 24:28])
                    wbeta = dncp.tile([BL, 1], F32, tag="wbeta")
                    beta_(wbeta[:], ifc[:, 34:35])
                    erase = dncp.tile([BL, WS], F32, tag="erase")
                    sigmoid_(erase[:], ifc[:, 35:41])

                    # pre-write M norms
                    msq = dncp.tile([BL, NW, WS], F32, tag="msq")
                    nc.vector.tensor_mul(msq[:], M_t[:], M_t[:])
                    mnr = dncp.tile([BL, NW], F32, tag="mnr")
                    nc.vector.reduce_sum(mnr[:], msq[:], axis=AX.X)
                    rnM = dncp.tile([BL, NW], F32, tag="rnM")
                    rsqrt_(rnM[:], mnr[:])
                    # wk norm
                    wks = dncp.tile([BL, WS], F32, tag="wks")
                    nc.vector.tensor_mul(wks[:], ifc[:, 28:34], ifc[:, 28:34])
                    wkn = dncp.tile([BL, 1], F32, tag="wkn")
                    nc.vector.reduce_sum(wkn[:], wks[:], axis=AX.X)
                    rnK = dncp.tile([BL, 1], F32, tag="rnK")
                    rsqrt_(rnK[:], wkn[:])
                    # write similarity + softmax (no max-sub; |z| bounded)
                    dp = dncp.tile([BL, NW, WS], F32, tag="dp")
                    nc.vector.tensor_mul(dp[:], M_t[:],
                                         _bc(ifc, BL, [(0, NW), (1, WS)], off=28))
                    dot = dncp.tile([BL, NW], F32, tag="dot")
                    nc.vector.reduce_sum(dot[:], dp[:], axis=AX.X)
                    nc.vector.tensor_mul(dot[:], dot[:], rnM[:])
                    nc.vector.tensor_scalar_mul(dot[:], dot[:], rnK[:])
                    ex = dncp.tile([BL, NW], F32, tag="ex")
                    exs = dncp.tile([BL, 1], F32, tag="exs")
                    nc.scalar.activation(ex[:], dot[:], AF.Exp, scale=wbeta[:],
                                         accum_out=exs[:])
                    rZ = dncp.tile([BL, 1], F32, tag="rZ")
                    nc.vector.reciprocal(rZ[:], exs[:])
                    ww = dncp.tile([BL, NW], F32, tag="ww")
                    nc.vector.tensor_scalar_mul(ww[:], ex[:], rZ[:])
                    # memory write: M = M*(1 - ww x er) + ww x wv
                    ewt = ewp.tile([BL, NW, WS], F32, tag="ewt")
                    nc.vector.tensor_mul(ewt[:], _bc(ww, BL, [(1, NW), (0, WS)]),
                                         _bc(erase, BL, [(0, NW), (1, WS)]))
                    nc.vector.tensor_scalar(ewt[:], ewt[:], -1.0, 1.0,
                                            op0=mybir.AluOpType.mult,
                                            op1=mybir.AluOpType.add)
                    nc.vector.tensor_mul(M_t[:], M_t[:], ewt[:])
                    wwv = ewp.tile([BL, NW, WS], F32, tag="wwv")
                    nc.vector.tensor_mul(wwv[:], _bc(ww, BL, [(1, NW), (0, WS)]),
                                         _bc(ifc, BL, [(0, NW), (1, WS)], off=41))
                    nc.vector.tensor_add(M_t[:], M_t[:], wwv[:])
                    # post-write norms
                    nc.vector.tensor_mul(msq[:], M_t[:], M_t[:])
                    mnr2 = dncp.tile([BL, NW], F32, tag="mnr2")
                    nc.vector.reduce_sum(mnr2[:], msq[:], axis=AX.X)
                    rnM2 = dncp.tile([BL, NW], F32, tag="rnM2")
                    rsqrt_(rnM2[:], mnr2[:])
                    # read keys norms
                    rks = dncp.tile([BL, R, WS], F32, tag="rks")
                    nc.vector.tensor_mul(rks[:], ifc[:, 0:24].rearrange("p (r w) -> p r w", r=R),
                                         ifc[:, 0:24].rearrange("p (r w) -> p r w", r=R))
                    rkn = dncp.tile([BL, R], F32, tag="rkn")
                    nc.vector.reduce_sum(rkn[:], rks[:], axis=AX.X)
                    rnR = dncp.tile([BL, R], F32, tag="rnR")
                    rsqrt_(rnR[:], rkn[:])
                    nc.vector.tensor_mul(rnR[:], rnR[:], rbeta[:])  # fold beta
                    # read scores [b, r, n]
                    pr2 = ewp.tile([BL, R, NW, WS], F32, tag="pr2")
                    nc.vector.tensor_mul(
                        pr2[:],
                        _bc(ifc, BL, [(WS, R), (0, NW), (1, WS)], off=0),
                        _bc(M_t, BL, [(0, R), (WS, NW), (1, WS)]))
                    sc = dncp.tile([BL, R, NW], F32, tag="sc")
                    nc.vector.reduce_sum(sc[:], pr2[:], axis=AX.X)
                    nc.vector.tensor_mul(sc[:], sc[:],
                                         _bc(rnM2, BL, [(0, R), (1, NW)]))
                    nc.vector.tensor_mul(sc[:], sc[:],
                                         _bc(rnR, BL, [(1, R), (0, NW)]))
                    e2 = dncp.tile([BL, R, NW], F32, tag="e2")
                    nc.scalar.activation(e2[:], sc[:], AF.Exp)
                    s2 = dncp.tile([BL, R], F32, tag="s2")
                    nc.vector.reduce_sum(s2[:], e2[:], axis=AX.X)
                    rS = dncp.tile([BL, R], F32, tag="rS")
                    nc.vector.reciprocal(rS[:], s2[:])
                    nc.vector.tensor_mul(e2[:], e2[:],
                                         _bc(rS, BL, [(1, R), (0, NW)]))
                    # r_vec [b, r, w] = sum_n w_r * M
                    pr3 = ewp.tile([BL, R, WS, NW], F32, tag="pr3")
                    nc.vector.tensor_mul(
                        pr3[:],
                        _bc(e2, BL, [(NW, R), (0, WS), (1, NW)]),
                        _bc(M_t, BL, [(0, R), (1, WS), (WS, NW)]))
                    rvec = dncp.tile([BL, RW], F32, tag="rvec")
                    nc.vector.reduce_sum(rvec[:], pr3[:], axis=AX.X)
                    rT_ps = ptrp.tile([RW, BL], F32, tag="tp")
                    nc.tensor.transpose(rT_ps[:], rvec[:], ident[:BL, :BL])
                    rT_new = rTp.tile([RW, BL], BF16, tag="rT")
                    nc.vector.tensor_copy(rT_new[:], rT_ps[:])
                    if dbg:
                        db = dncp.tile([BL, 384], F32, tag="db")
                        nc.vector.memset(db[:], 0.0)
                        nc.vector.tensor_copy(db[:, 328:336], ifc_sb[0:BL, :])
                        nc.vector.tensor_copy(db[:, 336:344], psIF[0:BL, :])
                        nc.vector.tensor_copy(db[:, 0:128], ifc[:])
                        nc.vector.tensor_copy(db[:, 128:153], ww[:])
                        nc.vector.tensor_copy(db[:, 153:177], rvec[:])
                        nc.vector.tensor_copy(db[:, 177:327],
                                              M_t[:].rearrange("p n w -> p (n w)"))
                        nc.sync.dma_start(out=dbg_t[t], in_=db[:])

                    hT_prev, rT_prev = hT_new, rT_new

                    # --- output staging flush every tblk steps ---
                    if toff == tblk - 1:
                        t0 = t - tblk + 1
                        for j in range(NCH):
                            otp = ptrp.tile([128, 128], F32, tag="tp")
                            nc.tensor.transpose(
                                otp[0:tblk * BL, :],
                                _bc(stage, 128, [(1, tblk * BL)],
                                    off=j * tblk * BL), ident[:])
                            ob = obp.tile([tblk * BL, 128], F32, tag="ob")
                            nc.scalar.copy(ob[:], otp[0:tblk * BL, :])
                            nc.sync.dma_start(
                                out=out_t[t0:t0 + tblk, :, 128 * j:128 * (j + 1)],
                                in_=ob[:])
                    if soff == sblk - 1 and blk + 1 < nblk:
                        xb_cur = xb_next

    nc.compile()
    return nc


_CACHE = {}


def _get_nc(seq, sblk, tblk):
    key = (seq, sblk, tblk)
    if key not in _CACHE:
        _CACHE[key] = _build(seq, sblk, tblk)
    return _CACHE[key]


def _prep_weights(W_ih, W_hh, b, W_if, b_if):
    W_ih = np.asarray(W_ih, np.float32)
    W_hh = np.asarray(W_hh, np.float32)
    b = np.asarray(b, np.float32)
    W_if = np.asarray(W_if, np.float32)
    b_if = np.asarray(b_if, np.float32)
    bigW = np.zeros((GROWS, KCOLS), np.float32)
    bigb = np.zeros((GROWS,), np.float32)
    # m=0: iface rows (h part only)
    bigW[0:IFACE, 512:1536] = W_if
    bigb[0:IFACE] = b_if
    # gate tiles: order (i, f, o, g) per hidden chunk
    gate_off = {0: 0, 1: HID, 2: 3 * HID, 3: 2 * HID}  # i, f, o, g
    for j in range(NCH):
        for gi in range(4):
            m = 1 + 4 * j + gi
            rows = slice(gate_off[gi] + 128 * j, gate_off[gi] + 128 * j + 128)
            bigW[128 * m:128 * m + 128, 0:512] = W_ih[rows, 0:512]
            bigW[128 * m:128 * m + 128, 512:1536] = W_hh[rows, :]
            bigW[128 * m:128 * m + 128, 1536:1536 + RW] = W_ih[rows, 512:512 + RW]
            bigb[128 * m:128 * m + 128] = b[rows]
    wt_host = np.ascontiguousarray(
        bigW.reshape(NMT, 128, NKT, 128).transpose(3, 2, 0, 1)
    ).astype(np.float16)
    bv_host = np.ascontiguousarray(bigb.reshape(NMT, 128).T)
    return wt_host, bv_host


def kernel(src, enc_pad_ix, emb_table, W_ih, W_hh, b, W_if, b_if,
           _seq=None, _sblk=64, _tblk=16, _trace=False):
    src = np.asarray(src)
    seq = src.shape[0] if _seq is None else _seq
    emb_np = np.asarray(emb_table).astype(np.float16)
    wt_host, bv_host = _prep_weights(W_ih, W_hh, b, W_if, b_if)

    nblk = seq // _sblk
    nsub = (_sblk * BL) // 128
    in_maps = []
    for c in range(NCORES):
        toks = np.ascontiguousarray(
            src[:seq, 8 * c:8 * c + 8].astype(np.int32)
            .reshape(nblk, _sblk * BL)
            .reshape(nblk, nsub, 128)
            .transpose(0, 2, 1))
        in_maps.append({"emb": emb_np, "wt": wt_host, "bv": bv_host,
                        "gidx": np.ascontiguousarray(toks)})

    nc = _get_nc(seq, _sblk, _tblk)
    import time as _time
    _t0 = _time.time()
    try:
        res = run_bass_kernel_spmd(nc, in_maps, core_ids=list(range(NCORES)),
                                   trace=_trace)
    except ModuleNotFoundError:
        res = run_bass_kernel_spmd(nc, in_maps, core_ids=list(range(NCORES)),
                                   trace=False)
    kernel._last_run_wall_s = _time.time() - _t0

    outputs = np.empty((seq, BATCH, HID), np.float32)
    for c in range(NCORES):
        outputs[:, 8 * c:8 * c + 8, :] = res.results[c]["out"]
    lengths = np.minimum((src[:seq] != int(enc_pad_ix)).sum(axis=0), seq - 1)
    last_state = outputs[lengths, np.arange(BATCH)]
    kernel._last_exec_time_ns = res.exec_time_ns
    return outputs, last_state
